# revision 6
# baseline (speedup 1.0000x reference)
"""Multi-head attention kernel for Trainium2 (Bass/Tile), 8-core SPMD.

Problem: B=4, L=S=2048, H=8, E=D=64, fp32.
  scores = einsum('blhe,bshe->bhls', Q, K) * tau[b] + delta[b]
  A = softmax(scores / sqrt(E), axis=-1)
  out = einsum('bhls,bshd->blhd', A, V)

Key observations:
  - softmax(a*x + c) == softmax(a*x): the per-batch delta bias cancels.
  - attn_mask is all-False / unused by the reference.
  - B*H = 32 (b,h) pairs, each an independent L x S attention block.
    Shard 4 pairs per core across 8 cores; no cross-core comms.

Per-core kernel design (per (b,h) pair):
  - Scores are computed TRANSPOSED: ST[s, l] chunks of [128, 512] so that
    the PV matmul can consume exp(ST) directly as the moving operand with
    full K=128 contraction (no P transposes).
  - QK: lhsT = K^T[e, s-chunk] (64x128), rhs = Q^T[e, l-tile] (64x512),
    out = ST chunk [128, 512] in PSUM. fp32r dtype -> 1 cycle/row.
  - exp: ScalarE activation Exp reading multi-bank PSUM groups, with the
    per-batch scale (tau[b]/sqrt(E)) folded into the activation scale.
  - PV: lhsT = V' chunk [s=128, 65] where column 64 is all-ones (computes
    softmax denominators for free), rhs = exp chunk [128, 512],
    accumulated over 16 s-chunks into O^T [65, 512] PSUM.
  - Tail: copy O^T to SBUF, PE-transpose 128-col blocks back to [128, 65],
    multiply by reciprocal of the denominator column, DMA out.

Host side only reshapes/slices (sharding + layout); all math is on-device.
"""

import os
import numpy as np

B, L, S, H, E = 4, 2048, 2048, 8, 64
NCORES = 8
NP = (B * H) // NCORES  # pairs per core = 4

LT = 512          # l-tile size (columns of ST chunks / PV moving dim)
NLT = L // LT     # 4
NSC = S // 128    # 16 s-chunks
# s-chunks per exp group; alternating 4/2 so adjacent groups use different
# PSUM pools (pipelining) while fitting the 8-bank PSUM budget:
# stA(4 banks) + stB(2) + O^T(1) + transpose(1) = 8.
GROUPS = (4, 2, 4, 2, 4)

_PROGRAM = None
LAST_RESULTS = None  # test harness reads exec_time_ns / trace path from here


def _build_program():
    import concourse.bass as bass
    import concourse.bacc as bacc
    import concourse.tile as tile
    from concourse import mybir
    from concourse.masks import make_identity

    f32 = mybir.dt.float32
    f32r = mybir.dt.float32r
    EXP = mybir.ActivationFunctionType.Exp

    nc = bacc.Bacc("TRN2", target_bir_lowering=False, debug=False,
                   num_devices=NCORES)
    qT = nc.dram_tensor("qt", [NP, E, L], f32, kind="ExternalInput").ap()
    kT = nc.dram_tensor("kt", [NP, E, S], f32, kind="ExternalInput").ap()
    v = nc.dram_tensor("v", [NP, S, E], f32, kind="ExternalInput").ap()
    taus = nc.dram_tensor("taus", [1, NP], f32, kind="ExternalInput").ap()
    o = nc.dram_tensor("o", [NP, L, E], f32, kind="ExternalOutput").ap()

    with tile.TileContext(nc) as tc:
        from contextlib import ExitStack
        with ExitStack() as ctx:
            consts = ctx.enter_context(tc.tile_pool(name="consts", bufs=1))
            kq_pool = ctx.enter_context(tc.tile_pool(name="kq", bufs=2))
            v_pool = ctx.enter_context(tc.tile_pool(name="vp", bufs=2))
            exp_pool = ctx.enter_context(tc.tile_pool(name="expp", bufs=3))
            tail_pool = ctx.enter_context(tc.tile_pool(name="tail", bufs=2))
            stA_pool = ctx.enter_context(
                tc.tile_pool(name="stA", bufs=1, space="PSUM"))
            stB_pool = ctx.enter_context(
                tc.tile_pool(name="stB", bufs=1, space="PSUM"))
            ot_pool = ctx.enter_context(
                tc.tile_pool(name="ot", bufs=1, space="PSUM"))
            tr_pool = ctx.enter_context(
                tc.tile_pool(name="tr", bufs=1, space="PSUM"))

            identity = consts.tile([128, 128], f32)
            make_identity(nc, identity)

            # tau[b] per pair, broadcast to all partitions; fold in 1/sqrt(E)
            tau_bc = consts.tile([128, NP], f32)
            nc.gpsimd.dma_start(out=tau_bc, in_=taus.to_broadcast([128, NP]))
            a_all = consts.tile([128, NP], f32)
            nc.scalar.mul(a_all, tau_bc, 1.0 / np.sqrt(float(E)))

            ones_col = consts.tile([128, 1], f32)
            nc.vector.memset(ones_col, 1.0)

            for p in range(NP):
                a_vec = a_all[:, p:p + 1]

                # f32r tiles; gpsimd DMA casts f32 -> f32r (rounds) in flight
                kt_sb = kq_pool.tile([64, S], f32r, tag="kt")
                nc.gpsimd.dma_start(out=kt_sb, in_=kT[p])
                qt_sb = kq_pool.tile([64, L], f32r, tag="qt")
                nc.gpsimd.dma_start(out=qt_sb, in_=qT[p])
                # V chunks [s=128, 65], col 64 = 1.0 (denominator trick)
                vp_sb = v_pool.tile([128, NSC, E + 1], f32r, tag="vp")
                nc.vector.tensor_copy(vp_sb[:, :, E:E + 1],
                                      ones_col.to_broadcast([128, NSC, 1]))
                nc.gpsimd.dma_start(
                    out=vp_sb[:, :, 0:E],
                    in_=v[p].rearrange("(n q) e -> q n e", q=128))

                for t in range(NLT):
                    ot_ps = ot_pool.tile([E + 1, LT], f32)
                    c0 = 0
                    for G in GROUPS:
                        pool = stA_pool if G == 4 else stB_pool
                        st_ps = pool.tile([128, G * LT], f32)
                        for k in range(G):
                            c = c0 + k
                            nc.tensor.matmul(
                                st_ps[:, k * LT:(k + 1) * LT],
                                lhsT=kt_sb[:, c * 128:(c + 1) * 128],
                                rhs=qt_sb[:, t * LT:(t + 1) * LT],
                                start=True, stop=True)
                        ex = exp_pool.tile([128, 4 * LT], f32r, tag="ex")
                        nc.scalar.activation(ex[:, 0:G * LT], st_ps, EXP,
                                             scale=a_vec)
                        for k in range(G):
                            c = c0 + k
                            nc.tensor.matmul(
                                ot_ps,
                                lhsT=vp_sb[:, c, :],
                                rhs=ex[:, k * LT:(k + 1) * LT],
                                start=(c == 0), stop=(c == NSC - 1))
                        c0 += G

                    # Tail: O^T [65, 512] -> normalized O [512, 64] in HBM
                    ot_sb = tail_pool.tile([E + 1, LT], f32, tag="otsb")
                    nc.vector.tensor_copy(ot_sb, ot_ps)
                    o_sb = tail_pool.tile([128, LT // 128, E], f32, tag="osb")
                    for j in range(LT // 128):
                        tr_ps = tr_pool.tile([128, E + 1], f32)
                        nc.tensor.transpose(
                            tr_ps, ot_sb[:, j * 128:(j + 1) * 128],
                            identity[0:E + 1, 0:E + 1])
                        rs = tail_pool.tile([128, 1], f32, tag="rs")
                        nc.vector.reciprocal(rs, tr_ps[:, E:E + 1])
                        nc.vector.tensor_scalar_mul(
                            o_sb[:, j, :], tr_ps[:, 0:E], rs)
                    nc.sync.dma_start(
                        out=o[p, t * LT:(t + 1) * LT, :].rearrange(
                            "(n q) e -> q n e", q=128),
                        in_=o_sb)
    nc.compile()
    return nc


def _get_program():
    global _PROGRAM
    if _PROGRAM is None:
        _PROGRAM = _build_program()
    return _PROGRAM


def kernel(queries, keys, values, attn_mask=None, tau=None, delta=None):
    from concourse.bass_utils import run_bass_kernel_spmd

    queries = np.ascontiguousarray(np.asarray(queries, dtype=np.float32))
    keys = np.ascontiguousarray(np.asarray(keys, dtype=np.float32))
    values = np.ascontiguousarray(np.asarray(values, dtype=np.float32))
    tau_flat = np.asarray(tau, dtype=np.float32).reshape(B)

    # pair = b*H + h; per-pair transposed layouts (host does layout only)
    qT_all = np.ascontiguousarray(
        queries.transpose(0, 2, 3, 1).reshape(B * H, E, L))
    kT_all = np.ascontiguousarray(
        keys.transpose(0, 2, 3, 1).reshape(B * H, E, S))
    v_all = np.ascontiguousarray(
        values.transpose(0, 2, 1, 3).reshape(B * H, S, E))

    nc = _get_program()
    in_maps = []
    for c in range(NCORES):
        lo = c * NP
        tau_pairs = np.ascontiguousarray(
            tau_flat[[(lo + i) // H for i in range(NP)]].reshape(1, NP))
        in_maps.append({
            "qt": qT_all[lo:lo + NP],
            "kt": kT_all[lo:lo + NP],
            "v": v_all[lo:lo + NP],
            "taus": tau_pairs,
        })

    kwargs = {}
    if os.environ.get("ATTN_TRACE"):
        kwargs["trace"] = True
        tmpdir = os.environ.get("ATTN_TRACE_DIR")
        if tmpdir:
            os.makedirs(tmpdir, exist_ok=True)
            kwargs["tmpdir"] = tmpdir

    res = run_bass_kernel_spmd(nc, in_maps, list(range(NCORES)), **kwargs)
    global LAST_RESULTS
    LAST_RESULTS = res

    o_all = np.concatenate([r["o"] for r in res.results], axis=0)  # [32, L, E]
    out = o_all.reshape(B, H, L, E).transpose(0, 2, 1, 3)  # [B, L, H, E]
    return np.ascontiguousarray(out)


# revision 7
# speedup vs baseline: 1.0460x; 1.0460x over previous
"""Multi-head attention kernel for Trainium2 (Bass/Tile), 8-core SPMD.

Problem: B=4, L=S=2048, H=8, E=D=64, fp32.
  scores = einsum('blhe,bshe->bhls', Q, K) * tau[b] + delta[b]
  A = softmax(scores / sqrt(E), axis=-1)
  out = einsum('bhls,bshd->blhd', A, V)

Key observations:
  - softmax(a*x + c) == softmax(a*x): the per-batch delta bias cancels.
  - attn_mask is all-False / unused by the reference.
  - B*H = 32 (b,h) pairs, each an independent L x S attention block.
    Shard 4 pairs per core across 8 cores; no cross-core comms.

Per-core kernel design (per (b,h) pair):
  - Scores are computed TRANSPOSED: ST[s, l] chunks of [128, 512] so that
    the PV matmul can consume exp(ST) directly as the moving operand with
    full K=128 contraction (no P transposes).
  - QK: lhsT = K^T[e, s-chunk] (64x128), rhs = Q^T[e, l-tile] (64x512),
    out = ST chunk [128, 512] in PSUM. fp32r dtype -> 1 cycle/row.
  - exp: ScalarE activation Exp reading multi-bank PSUM groups, with the
    per-batch scale (tau[b]/sqrt(E)) folded into the activation scale.
  - PV: lhsT = V' chunk [s=128, 65] where column 64 is all-ones (computes
    softmax denominators for free), rhs = exp chunk [128, 512],
    accumulated over 16 s-chunks into O^T [65, 512] PSUM.
  - Tail: copy O^T to SBUF, PE-transpose 128-col blocks back to [128, 65],
    multiply by reciprocal of the denominator column, DMA out.

Host side only reshapes/slices (sharding + layout); all math is on-device.
"""

import os
import numpy as np

B, L, S, H, E = 4, 2048, 2048, 8, 64
NCORES = 8
NP = (B * H) // NCORES  # pairs per core = 4

LT = 512          # l-tile size (columns of ST chunks / PV moving dim)
NLT = L // LT     # 4
NSC = S // 128    # 16 s-chunks
# s-chunks per exp group; alternating 4/2 so adjacent groups use different
# PSUM pools (pipelining) while fitting the 8-bank PSUM budget:
# stA(4 banks) + stB(2) + O^T(1) + transpose(1) = 8.
GROUPS = (4, 2, 4, 2, 4)

_PROGRAM = None
LAST_RESULTS = None  # test harness reads exec_time_ns / trace path from here


def _build_program():
    import concourse.bass as bass
    import concourse.bacc as bacc
    import concourse.tile as tile
    from concourse import mybir
    from concourse.masks import make_identity

    f32 = mybir.dt.float32
    f16 = mybir.dt.float16
    EXP = mybir.ActivationFunctionType.Exp

    nc = bacc.Bacc("TRN2", target_bir_lowering=False, debug=False,
                   num_devices=NCORES)
    qT = nc.dram_tensor("qt", [NP, E, L], f32, kind="ExternalInput").ap()
    kT = nc.dram_tensor("kt", [NP, E, S], f32, kind="ExternalInput").ap()
    v = nc.dram_tensor("v", [NP, S, E], f32, kind="ExternalInput").ap()
    taus = nc.dram_tensor("taus", [1, NP], f32, kind="ExternalInput").ap()
    o = nc.dram_tensor("o", [NP, L, E], f32, kind="ExternalOutput").ap()

    with tile.TileContext(nc) as tc:
        from contextlib import ExitStack
        with ExitStack() as ctx:
            consts = ctx.enter_context(tc.tile_pool(name="consts", bufs=1))
            kq_pool = ctx.enter_context(tc.tile_pool(name="kq", bufs=2))
            v_pool = ctx.enter_context(tc.tile_pool(name="vp", bufs=2))
            exp_pool = ctx.enter_context(tc.tile_pool(name="expp", bufs=3))
            tail_pool = ctx.enter_context(tc.tile_pool(name="tail", bufs=2))
            stA_pool = ctx.enter_context(
                tc.tile_pool(name="stA", bufs=1, space="PSUM"))
            stB_pool = ctx.enter_context(
                tc.tile_pool(name="stB", bufs=1, space="PSUM"))
            ot_pool = ctx.enter_context(
                tc.tile_pool(name="ot", bufs=1, space="PSUM"))
            tr_pool = ctx.enter_context(
                tc.tile_pool(name="tr", bufs=1, space="PSUM"))

            identity = consts.tile([128, 128], f32)
            make_identity(nc, identity)

            # tau[b] per pair, broadcast to all partitions; fold in 1/sqrt(E)
            tau_bc = consts.tile([128, NP], f32)
            nc.gpsimd.dma_start(out=tau_bc, in_=taus.to_broadcast([128, NP]))
            a_all = consts.tile([128, NP], f32)
            nc.scalar.mul(a_all, tau_bc, 1.0 / np.sqrt(float(E)))

            ones_col = consts.tile([128, 1], f32)
            nc.vector.memset(ones_col, 1.0)

            for p in range(NP):
                a_vec = a_all[:, p:p + 1]

                # fp16 operands (1 cyc/row PE stream + fast weight load);
                # gpsimd DMA casts f32 -> f16 in flight
                kt_sb = kq_pool.tile([64, S], f16, tag="kt")
                nc.gpsimd.dma_start(out=kt_sb, in_=kT[p])
                qt_sb = kq_pool.tile([64, L], f16, tag="qt")
                nc.gpsimd.dma_start(out=qt_sb, in_=qT[p])
                # V chunks [s=128, 65], col 64 = 1.0 (denominator trick)
                vp_sb = v_pool.tile([128, NSC, E + 1], f16, tag="vp")
                nc.vector.tensor_copy(vp_sb[:, :, E:E + 1],
                                      ones_col.to_broadcast([128, NSC, 1]))
                nc.gpsimd.dma_start(
                    out=vp_sb[:, :, 0:E],
                    in_=v[p].rearrange("(n q) e -> q n e", q=128))

                for t in range(NLT):
                    ot_ps = ot_pool.tile([E + 1, LT], f32)
                    c0 = 0
                    for G in GROUPS:
                        pool = stA_pool if G == 4 else stB_pool
                        st_ps = pool.tile([128, G * LT], f32)
                        for k in range(G):
                            c = c0 + k
                            nc.tensor.matmul(
                                st_ps[:, k * LT:(k + 1) * LT],
                                lhsT=kt_sb[:, c * 128:(c + 1) * 128],
                                rhs=qt_sb[:, t * LT:(t + 1) * LT],
                                start=True, stop=True)
                        ex = exp_pool.tile([128, 4 * LT], f16, tag="ex")
                        nc.scalar.activation(ex[:, 0:G * LT], st_ps, EXP,
                                             scale=a_vec)
                        for k in range(G):
                            c = c0 + k
                            nc.tensor.matmul(
                                ot_ps,
                                lhsT=vp_sb[:, c, :],
                                rhs=ex[:, k * LT:(k + 1) * LT],
                                start=(c == 0), stop=(c == NSC - 1))
                        c0 += G

                    # Tail: O^T [65, 512] -> normalized O [512, 64] in HBM
                    ot_sb = tail_pool.tile([E + 1, LT], f32, tag="otsb")
                    nc.vector.tensor_copy(ot_sb, ot_ps)
                    o_sb = tail_pool.tile([128, LT // 128, E], f32, tag="osb")
                    for j in range(LT // 128):
                        tr_ps = tr_pool.tile([128, E + 1], f32)
                        nc.tensor.transpose(
                            tr_ps, ot_sb[:, j * 128:(j + 1) * 128],
                            identity[0:E + 1, 0:E + 1])
                        rs = tail_pool.tile([128, 1], f32, tag="rs")
                        nc.vector.reciprocal(rs, tr_ps[:, E:E + 1])
                        nc.vector.tensor_scalar_mul(
                            o_sb[:, j, :], tr_ps[:, 0:E], rs)
                    nc.sync.dma_start(
                        out=o[p, t * LT:(t + 1) * LT, :].rearrange(
                            "(n q) e -> q n e", q=128),
                        in_=o_sb)
    nc.compile()
    return nc


def _get_program():
    global _PROGRAM
    if _PROGRAM is None:
        _PROGRAM = _build_program()
    return _PROGRAM


def kernel(queries, keys, values, attn_mask=None, tau=None, delta=None):
    from concourse.bass_utils import run_bass_kernel_spmd

    queries = np.ascontiguousarray(np.asarray(queries, dtype=np.float32))
    keys = np.ascontiguousarray(np.asarray(keys, dtype=np.float32))
    values = np.ascontiguousarray(np.asarray(values, dtype=np.float32))
    tau_flat = np.asarray(tau, dtype=np.float32).reshape(B)

    # pair = b*H + h; per-pair transposed layouts (host does layout only)
    qT_all = np.ascontiguousarray(
        queries.transpose(0, 2, 3, 1).reshape(B * H, E, L))
    kT_all = np.ascontiguousarray(
        keys.transpose(0, 2, 3, 1).reshape(B * H, E, S))
    v_all = np.ascontiguousarray(
        values.transpose(0, 2, 1, 3).reshape(B * H, S, E))

    nc = _get_program()
    in_maps = []
    for c in range(NCORES):
        lo = c * NP
        tau_pairs = np.ascontiguousarray(
            tau_flat[[(lo + i) // H for i in range(NP)]].reshape(1, NP))
        in_maps.append({
            "qt": qT_all[lo:lo + NP],
            "kt": kT_all[lo:lo + NP],
            "v": v_all[lo:lo + NP],
            "taus": tau_pairs,
        })

    kwargs = {}
    if os.environ.get("ATTN_TRACE"):
        kwargs["trace"] = True
        tmpdir = os.environ.get("ATTN_TRACE_DIR")
        if tmpdir:
            os.makedirs(tmpdir, exist_ok=True)
            kwargs["tmpdir"] = tmpdir

    res = run_bass_kernel_spmd(nc, in_maps, list(range(NCORES)), **kwargs)
    global LAST_RESULTS
    LAST_RESULTS = res

    o_all = np.concatenate([r["o"] for r in res.results], axis=0)  # [32, L, E]
    out = o_all.reshape(B, H, L, E).transpose(0, 2, 1, 3)  # [B, L, H, E]
    return np.ascontiguousarray(out)


# revision 8
# speedup vs baseline: 1.1698x; 1.1184x over previous
"""Multi-head attention kernel for Trainium2 (Bass/Tile), 8-core SPMD.

Problem: B=4, L=S=2048, H=8, E=D=64, fp32.
  scores = einsum('blhe,bshe->bhls', Q, K) * tau[b] + delta[b]
  A = softmax(scores / sqrt(E), axis=-1)
  out = einsum('bhls,bshd->blhd', A, V)

Key observations:
  - softmax(a*x + c) == softmax(a*x): the per-batch delta bias cancels.
  - attn_mask is all-False / unused by the reference.
  - B*H = 32 (b,h) pairs, each an independent L x S attention block.
    Shard 4 pairs per core across 8 cores; no cross-core comms.

Per-core kernel design (per (b,h) pair):
  - Scores are computed TRANSPOSED: ST[s, l] chunks of [128, 512] so that
    the PV matmul can consume exp(ST) directly as the moving operand with
    full K=128 contraction (no P transposes).
  - QK: lhsT = K^T[e, s-chunk] (64x128), rhs = Q^T[e, l-tile] (64x512),
    out = ST chunk [128, 512] in PSUM. fp32r dtype -> 1 cycle/row.
  - exp: ScalarE activation Exp reading multi-bank PSUM groups, with the
    per-batch scale (tau[b]/sqrt(E)) folded into the activation scale.
  - PV: lhsT = V' chunk [s=128, 65] where column 64 is all-ones (computes
    softmax denominators for free), rhs = exp chunk [128, 512],
    accumulated over 16 s-chunks into O^T [65, 512] PSUM.
  - Tail: copy O^T to SBUF, PE-transpose 128-col blocks back to [128, 65],
    multiply by reciprocal of the denominator column, DMA out.

Host side only reshapes/slices (sharding + layout); all math is on-device.
"""

import os
import numpy as np

B, L, S, H, E = 4, 2048, 2048, 8, 64
NCORES = 8
NP = (B * H) // NCORES  # pairs per core = 4

LT = 512          # l-tile size (columns of ST chunks / PV moving dim)
NLT = L // LT     # 4
NSC = S // 128    # 16 s-chunks
# s-chunks per exp group; alternating 4/2 so adjacent groups use different
# PSUM pools (pipelining) while fitting the 8-bank PSUM budget:
# stA(4 banks) + stB(2) + O^T(1) + transpose(1) = 8.
GROUPS = (4, 2, 4, 2, 4)

_PROGRAM = None
LAST_RESULTS = None  # test harness reads exec_time_ns / trace path from here


def _build_program():
    import concourse.bass as bass
    import concourse.bacc as bacc
    import concourse.tile as tile
    from concourse import mybir
    from concourse.masks import make_identity

    f32 = mybir.dt.float32
    f16 = mybir.dt.float16
    EXP = mybir.ActivationFunctionType.Exp

    nc = bacc.Bacc("TRN2", target_bir_lowering=False, debug=False,
                   num_devices=NCORES)
    # qt2: Q^T duplicated on both partition halves [128, L].
    # kt2: K^T s-chunk pairs split across partition halves:
    #   kt2[0:64, j, :] = K^T chunk 2j, kt2[64:128, j, :] = chunk 2j+1.
    qT = nc.dram_tensor("qt", [NP, 2 * E, L], f32, kind="ExternalInput").ap()
    kT = nc.dram_tensor("kt", [NP, 2 * E, NSC // 2, 128], f32,
                        kind="ExternalInput").ap()
    v = nc.dram_tensor("v", [NP, S, E], f32, kind="ExternalInput").ap()
    taus = nc.dram_tensor("taus", [1, NP], f32, kind="ExternalInput").ap()
    o = nc.dram_tensor("o", [NP, L, E], f32, kind="ExternalOutput").ap()

    with tile.TileContext(nc) as tc:
        from contextlib import ExitStack
        with ExitStack() as ctx:
            consts = ctx.enter_context(tc.tile_pool(name="consts", bufs=1))
            kq_pool = ctx.enter_context(tc.tile_pool(name="kq", bufs=2))
            v_pool = ctx.enter_context(tc.tile_pool(name="vp", bufs=2))
            exp_pool = ctx.enter_context(tc.tile_pool(name="expp", bufs=3))
            tail_pool = ctx.enter_context(tc.tile_pool(name="tail", bufs=2))
            stA_pool = ctx.enter_context(
                tc.tile_pool(name="stA", bufs=1, space="PSUM"))
            stB_pool = ctx.enter_context(
                tc.tile_pool(name="stB", bufs=1, space="PSUM"))
            ot_pool = ctx.enter_context(
                tc.tile_pool(name="ot", bufs=1, space="PSUM"))
            tr_pool = ctx.enter_context(
                tc.tile_pool(name="tr", bufs=1, space="PSUM"))

            identity = consts.tile([128, 128], f32)
            make_identity(nc, identity)

            # tau[b] per pair, broadcast to all partitions; fold in 1/sqrt(E)
            tau_bc = consts.tile([128, NP], f32)
            nc.gpsimd.dma_start(out=tau_bc, in_=taus.to_broadcast([128, NP]))
            a_all = consts.tile([128, NP], f32)
            nc.scalar.mul(a_all, tau_bc, 1.0 / np.sqrt(float(E)))

            ones_col = consts.tile([128, 1], f32)
            nc.vector.memset(ones_col, 1.0)

            for p in range(NP):
                a_vec = a_all[:, p:p + 1]

                # fp16 operands (1 cyc/row PE stream + fast weight load);
                # gpsimd DMA casts f32 -> f16 in flight
                kt_sb = kq_pool.tile([128, NSC // 2, 128], f16, tag="kt")
                nc.gpsimd.dma_start(out=kt_sb, in_=kT[p])
                qt_sb = kq_pool.tile([128, L], f16, tag="qt")
                nc.gpsimd.dma_start(out=qt_sb, in_=qT[p])
                # V chunks [s=128, 65], col 64 = 1.0 (denominator trick)
                vp_sb = v_pool.tile([128, NSC, E + 1], f16, tag="vp")
                nc.vector.tensor_copy(vp_sb[:, :, E:E + 1],
                                      ones_col.to_broadcast([128, NSC, 1]))
                nc.gpsimd.dma_start(
                    out=vp_sb[:, :, 0:E],
                    in_=v[p].rearrange("(n q) e -> q n e", q=128))

                for t in range(NLT):
                    ot_ps = ot_pool.tile([E + 1, LT], f32)
                    c0 = 0
                    for G in GROUPS:
                        pool = stA_pool if G == 4 else stB_pool
                        st_ps = pool.tile([128, G * LT], f32)
                        for k2 in range(G // 2):
                            j = (c0 // 2) + k2  # packed chunk-pair index
                            nc.tensor.matmul(
                                st_ps[:, (2 * k2) * LT:(2 * k2 + 1) * LT],
                                lhsT=kt_sb[0:64, j, :],
                                rhs=qt_sb[0:64, t * LT:(t + 1) * LT],
                                start=True, stop=True,
                                tile_position=(0, 0))
                            nc.tensor.matmul(
                                st_ps[:, (2 * k2 + 1) * LT:(2 * k2 + 2) * LT],
                                lhsT=kt_sb[64:128, j, :],
                                rhs=qt_sb[64:128, t * LT:(t + 1) * LT],
                                start=True, stop=True,
                                tile_position=(64, 0))
                        ex = exp_pool.tile([128, 4 * LT], f16, tag="ex")
                        nc.scalar.activation(ex[:, 0:G * LT], st_ps, EXP,
                                             scale=a_vec)
                        for k in range(G):
                            c = c0 + k
                            nc.tensor.matmul(
                                ot_ps,
                                lhsT=vp_sb[:, c, :],
                                rhs=ex[:, k * LT:(k + 1) * LT],
                                start=(c == 0), stop=(c == NSC - 1))
                        c0 += G

                    # Tail: O^T [65, 512] -> normalized O [512, 64] in HBM
                    ot_sb = tail_pool.tile([E + 1, LT], f32, tag="otsb")
                    nc.vector.tensor_copy(ot_sb, ot_ps)
                    o_sb = tail_pool.tile([128, LT // 128, E], f32, tag="osb")
                    for j in range(LT // 128):
                        tr_ps = tr_pool.tile([128, E + 1], f32)
                        nc.tensor.transpose(
                            tr_ps, ot_sb[:, j * 128:(j + 1) * 128],
                            identity[0:E + 1, 0:E + 1])
                        rs = tail_pool.tile([128, 1], f32, tag="rs")
                        nc.vector.reciprocal(rs, tr_ps[:, E:E + 1])
                        nc.vector.tensor_scalar_mul(
                            o_sb[:, j, :], tr_ps[:, 0:E], rs)
                    nc.sync.dma_start(
                        out=o[p, t * LT:(t + 1) * LT, :].rearrange(
                            "(n q) e -> q n e", q=128),
                        in_=o_sb)
    nc.compile()
    return nc


def _get_program():
    global _PROGRAM
    if _PROGRAM is None:
        _PROGRAM = _build_program()
    return _PROGRAM


def kernel(queries, keys, values, attn_mask=None, tau=None, delta=None):
    from concourse.bass_utils import run_bass_kernel_spmd

    queries = np.ascontiguousarray(np.asarray(queries, dtype=np.float32))
    keys = np.ascontiguousarray(np.asarray(keys, dtype=np.float32))
    values = np.ascontiguousarray(np.asarray(values, dtype=np.float32))
    tau_flat = np.asarray(tau, dtype=np.float32).reshape(B)

    # pair = b*H + h; per-pair transposed layouts (host does layout only)
    qT_base = queries.transpose(0, 2, 3, 1).reshape(B * H, E, L)
    qT_all = np.ascontiguousarray(
        np.concatenate([qT_base, qT_base], axis=1))  # [32, 128, L] duplicated
    kT_base = keys.transpose(0, 2, 3, 1).reshape(B * H, E, S)
    kc = kT_base.reshape(B * H, E, S // 128, 128)
    kT_all = np.ascontiguousarray(
        np.concatenate([kc[:, :, 0::2, :], kc[:, :, 1::2, :]], axis=1))
    # kT_all: [32, 128, 8, 128]; rows 0:64 = even chunks, 64:128 = odd
    v_all = np.ascontiguousarray(
        values.transpose(0, 2, 1, 3).reshape(B * H, S, E))

    nc = _get_program()
    in_maps = []
    for c in range(NCORES):
        lo = c * NP
        tau_pairs = np.ascontiguousarray(
            tau_flat[[(lo + i) // H for i in range(NP)]].reshape(1, NP))
        in_maps.append({
            "qt": qT_all[lo:lo + NP],
            "kt": kT_all[lo:lo + NP],
            "v": v_all[lo:lo + NP],
            "taus": tau_pairs,
        })

    kwargs = {}
    if os.environ.get("ATTN_TRACE"):
        kwargs["trace"] = True
        tmpdir = os.environ.get("ATTN_TRACE_DIR")
        if tmpdir:
            os.makedirs(tmpdir, exist_ok=True)
            kwargs["tmpdir"] = tmpdir

    res = run_bass_kernel_spmd(nc, in_maps, list(range(NCORES)), **kwargs)
    global LAST_RESULTS
    LAST_RESULTS = res

    o_all = np.concatenate([r["o"] for r in res.results], axis=0)  # [32, L, E]
    out = o_all.reshape(B, H, L, E).transpose(0, 2, 1, 3)  # [B, L, H, E]
    return np.ascontiguousarray(out)


# revision 10
# speedup vs baseline: 1.2132x; 1.0371x over previous
"""Multi-head attention kernel for Trainium2 (Bass/Tile), 8-core SPMD.

Problem: B=4, L=S=2048, H=8, E=D=64, fp32.
  scores = einsum('blhe,bshe->bhls', Q, K) * tau[b] + delta[b]
  A = softmax(scores / sqrt(E), axis=-1)
  out = einsum('bhls,bshd->blhd', A, V)

Key observations:
  - softmax(a*x + c) == softmax(a*x): the per-batch delta bias cancels.
  - attn_mask is all-False / unused by the reference.
  - B*H = 32 (b,h) pairs, each an independent L x S attention block.
    Shard 4 pairs per core across 8 cores; no cross-core comms.

Per-core kernel design (per (b,h) pair):
  - Scores are computed TRANSPOSED: ST[s, l] chunks of [128, 512] so that
    the PV matmul can consume exp(ST) directly as the moving operand with
    full K=128 contraction (no P transposes).
  - QK: lhsT = K^T[e, s-chunk] (64x128), rhs = Q^T[e, l-tile] (64x512),
    out = ST chunk [128, 512] in PSUM. fp32r dtype -> 1 cycle/row.
  - exp: ScalarE activation Exp reading multi-bank PSUM groups, with the
    per-batch scale (tau[b]/sqrt(E)) folded into the activation scale.
  - PV: lhsT = V' chunk [s=128, 65] where column 64 is all-ones (computes
    softmax denominators for free), rhs = exp chunk [128, 512],
    accumulated over 16 s-chunks into O^T [65, 512] PSUM.
  - Tail: copy O^T to SBUF, PE-transpose 128-col blocks back to [128, 65],
    multiply by reciprocal of the denominator column, DMA out.

Host side only reshapes/slices (sharding + layout); all math is on-device.
"""

import os
import numpy as np

B, L, S, H, E = 4, 2048, 2048, 8, 64
NCORES = 8
NP = (B * H) // NCORES  # pairs per core = 4

LT = 512          # l-tile size (columns of ST chunks / PV moving dim)
NLT = L // LT     # 4
NSC = S // 128    # 16 s-chunks
# s-chunks per exp group; alternating 4/2 so adjacent groups use different
# PSUM pools (pipelining) while fitting the 8-bank PSUM budget:
# stA(4 banks) + stB(2) + O^T(1) + transpose(1) = 8.
GROUPS = (4, 2, 4, 2, 4)

_PROGRAM = None
LAST_RESULTS = None  # test harness reads exec_time_ns / trace path from here


def _build_program():
    import concourse.bass as bass
    import concourse.bacc as bacc
    import concourse.tile as tile
    from concourse import mybir

    f32 = mybir.dt.float32
    f16 = mybir.dt.float16
    EXP = mybir.ActivationFunctionType.Exp

    nc = bacc.Bacc("TRN2", target_bir_lowering=False, debug=False,
                   num_devices=NCORES)
    # qt2: Q^T duplicated on both partition halves [128, L].
    # kt2: K^T s-chunk pairs split across partition halves:
    #   kt2[0:64, j, :] = K^T chunk 2j, kt2[64:128, j, :] = chunk 2j+1.
    qT = nc.dram_tensor("qt", [NP, 2 * E, L], f32, kind="ExternalInput").ap()
    kT = nc.dram_tensor("kt", [NP, 2 * E, NSC // 2, 128], f32,
                        kind="ExternalInput").ap()
    v = nc.dram_tensor("v", [NP, S, E], f32, kind="ExternalInput").ap()
    taus = nc.dram_tensor("taus", [1, NP], f32, kind="ExternalInput").ap()
    o = nc.dram_tensor("o", [NP, E, L], f32, kind="ExternalOutput").ap()

    with tile.TileContext(nc) as tc:
        from contextlib import ExitStack
        with ExitStack() as ctx:
            consts = ctx.enter_context(tc.tile_pool(name="consts", bufs=1))
            kq_pool = ctx.enter_context(tc.tile_pool(name="kq", bufs=2))
            v_pool = ctx.enter_context(tc.tile_pool(name="vp", bufs=2))
            exp_pool = ctx.enter_context(tc.tile_pool(name="expp", bufs=3))
            tail_pool = ctx.enter_context(tc.tile_pool(name="tail", bufs=2))
            stA_pool = ctx.enter_context(
                tc.tile_pool(name="stA", bufs=1, space="PSUM"))
            stB_pool = ctx.enter_context(
                tc.tile_pool(name="stB", bufs=1, space="PSUM"))
            ot_pool = ctx.enter_context(
                tc.tile_pool(name="ot", bufs=2, space="PSUM"))

            # tau[b] per pair, broadcast to all partitions; fold in 1/sqrt(E)
            tau_bc = consts.tile([128, NP], f32)
            nc.gpsimd.dma_start(out=tau_bc, in_=taus.to_broadcast([128, NP]))
            a_all = consts.tile([128, NP], f32)
            nc.scalar.mul(a_all, tau_bc, 1.0 / np.sqrt(float(E)))

            ones_col = consts.tile([128, 1], f32)
            nc.vector.memset(ones_col, 1.0)

            for p in range(NP):
                a_vec = a_all[:, p:p + 1]

                # fp16 operands (1 cyc/row PE stream + fast weight load);
                # gpsimd DMA casts f32 -> f16 in flight
                kt_sb = kq_pool.tile([128, NSC // 2, 128], f16, tag="kt")
                nc.gpsimd.dma_start(out=kt_sb, in_=kT[p])
                qt_sb = kq_pool.tile([128, L], f16, tag="qt")
                nc.gpsimd.dma_start(out=qt_sb, in_=qT[p])
                # V chunks [s=128, 65], col 64 = 1.0 (denominator trick)
                vp_sb = v_pool.tile([128, NSC, E + 1], f16, tag="vp")
                nc.vector.tensor_copy(vp_sb[:, :, E:E + 1],
                                      ones_col.to_broadcast([128, NSC, 1]))
                nc.gpsimd.dma_start(
                    out=vp_sb[:, :, 0:E],
                    in_=v[p].rearrange("(n q) e -> q n e", q=128))

                for t in range(NLT):
                    ot_ps = ot_pool.tile([E + 1, LT], f32)
                    c0 = 0
                    for G in GROUPS:
                        pool = stA_pool if G == 4 else stB_pool
                        st_ps = pool.tile([128, G * LT], f32)
                        for k2 in range(G // 2):
                            j = (c0 // 2) + k2  # packed chunk-pair index
                            nc.tensor.matmul(
                                st_ps[:, (2 * k2) * LT:(2 * k2 + 1) * LT],
                                lhsT=kt_sb[0:64, j, :],
                                rhs=qt_sb[0:64, t * LT:(t + 1) * LT],
                                start=True, stop=True,
                                tile_position=(0, 0))
                            nc.tensor.matmul(
                                st_ps[:, (2 * k2 + 1) * LT:(2 * k2 + 2) * LT],
                                lhsT=kt_sb[64:128, j, :],
                                rhs=qt_sb[64:128, t * LT:(t + 1) * LT],
                                start=True, stop=True,
                                tile_position=(64, 0))
                        ex = exp_pool.tile([128, 4 * LT], f16, tag="ex")
                        nc.scalar.activation(ex[:, 0:G * LT], st_ps, EXP,
                                             scale=a_vec)
                        for k in range(G):
                            c = c0 + k
                            nc.tensor.matmul(
                                ot_ps,
                                lhsT=vp_sb[:, c, :],
                                rhs=ex[:, k * LT:(k + 1) * LT],
                                start=(c == 0), stop=(c == NSC - 1))
                        c0 += G

                    # Tail: normalize O^T by its denominator row and
                    # store O^T; host transposes back. recip row is
                    # partition-broadcast via DMA replication.
                    ot_sb = tail_pool.tile([E + 1, LT], f32, tag="otsb")
                    nc.vector.tensor_copy(ot_sb, ot_ps)
                    rs_row = tail_pool.tile([1, LT], f32, tag="rsrow")
                    nc.vector.reciprocal(rs_row, ot_sb[E:E + 1, :])
                    rb = tail_pool.tile([64, LT], f32, tag="rb")
                    nc.gpsimd.partition_broadcast(rb, rs_row, channels=64)
                    on = tail_pool.tile([64, LT], f32, tag="on")
                    nc.vector.tensor_mul(on, ot_sb[0:E, :], rb)
                    nc.sync.dma_start(out=o[p, :, t * LT:(t + 1) * LT],
                                      in_=on)
    nc.compile()
    return nc


def _get_program():
    global _PROGRAM
    if _PROGRAM is None:
        _PROGRAM = _build_program()
    return _PROGRAM


def kernel(queries, keys, values, attn_mask=None, tau=None, delta=None):
    from concourse.bass_utils import run_bass_kernel_spmd

    queries = np.ascontiguousarray(np.asarray(queries, dtype=np.float32))
    keys = np.ascontiguousarray(np.asarray(keys, dtype=np.float32))
    values = np.ascontiguousarray(np.asarray(values, dtype=np.float32))
    tau_flat = np.asarray(tau, dtype=np.float32).reshape(B)

    # pair = b*H + h; per-pair transposed layouts (host does layout only)
    qT_base = queries.transpose(0, 2, 3, 1).reshape(B * H, E, L)
    qT_all = np.ascontiguousarray(
        np.concatenate([qT_base, qT_base], axis=1))  # [32, 128, L] duplicated
    kT_base = keys.transpose(0, 2, 3, 1).reshape(B * H, E, S)
    kc = kT_base.reshape(B * H, E, S // 128, 128)
    kT_all = np.ascontiguousarray(
        np.concatenate([kc[:, :, 0::2, :], kc[:, :, 1::2, :]], axis=1))
    # kT_all: [32, 128, 8, 128]; rows 0:64 = even chunks, 64:128 = odd
    v_all = np.ascontiguousarray(
        values.transpose(0, 2, 1, 3).reshape(B * H, S, E))

    nc = _get_program()
    in_maps = []
    for c in range(NCORES):
        lo = c * NP
        tau_pairs = np.ascontiguousarray(
            tau_flat[[(lo + i) // H for i in range(NP)]].reshape(1, NP))
        in_maps.append({
            "qt": qT_all[lo:lo + NP],
            "kt": kT_all[lo:lo + NP],
            "v": v_all[lo:lo + NP],
            "taus": tau_pairs,
        })

    kwargs = {}
    if os.environ.get("ATTN_TRACE"):
        kwargs["trace"] = True
        tmpdir = os.environ.get("ATTN_TRACE_DIR")
        if tmpdir:
            os.makedirs(tmpdir, exist_ok=True)
            kwargs["tmpdir"] = tmpdir

    res = run_bass_kernel_spmd(nc, in_maps, list(range(NCORES)), **kwargs)
    global LAST_RESULTS
    LAST_RESULTS = res

    o_all = np.concatenate([r["o"] for r in res.results], axis=0)  # [32, E, L]
    out = o_all.reshape(B, H, E, L).transpose(0, 3, 1, 2)  # [B, L, H, E]
    return np.ascontiguousarray(out)


# revision 13
# speedup vs baseline: 1.8694x; 1.5409x over previous
"""Multi-head attention kernel for Trainium2 (Bass/Tile), 8-core SPMD.

Problem: B=4, L=S=2048, H=8, E=D=64, fp32.
  scores = einsum('blhe,bshe->bhls', Q, K) * tau[b] + delta[b]
  A = softmax(scores / sqrt(E), axis=-1)
  out = einsum('bhls,bshd->blhd', A, V)

Key observations:
  - softmax(a*x + c) == softmax(a*x): the per-batch delta bias cancels.
  - attn_mask is all-False / unused by the reference.
  - B*H = 32 (b,h) pairs, each an independent L x S attention block.
    Shard 4 pairs per core across 8 cores; no cross-core comms.

Per-core kernel design (per (b,h) pair):
  - Scores are computed TRANSPOSED: ST[s, l] chunks of [128, 512] so that
    the PV matmul can consume exp(ST) directly as the moving operand with
    full K=128 contraction (no P transposes).
  - QK: lhsT = K^T[e, s-chunk] (64x128), rhs = Q^T[e, l-tile] (64x512),
    out = ST chunk [128, 512] in PSUM. fp32r dtype -> 1 cycle/row.
  - exp: ScalarE activation Exp reading multi-bank PSUM groups, with the
    per-batch scale (tau[b]/sqrt(E)) folded into the activation scale.
  - PV: lhsT = V' chunk [s=128, 65] where column 64 is all-ones (computes
    softmax denominators for free), rhs = exp chunk [128, 512],
    accumulated over 16 s-chunks into O^T [65, 512] PSUM.
  - Tail: copy O^T to SBUF, PE-transpose 128-col blocks back to [128, 65],
    multiply by reciprocal of the denominator column, DMA out.

Host side only reshapes/slices (sharding + layout); all math is on-device.
"""

import os
import numpy as np

B, L, S, H, E = 4, 2048, 2048, 8, 64
NCORES = 8
NP = (B * H) // NCORES  # pairs per core = 4

LT = 512          # l-tile size (columns of ST chunks / PV moving dim)
NLT = L // LT     # 4
NSC = S // 128    # 16 s-chunks
# s-chunks per exp group; alternating 4/2 so adjacent groups use different
# PSUM pools (pipelining) while fitting the 8-bank PSUM budget:
# stA(4 banks) + stB(2) + O^T(1) + transpose(1) = 8.
GROUPS = (4, 2, 4, 2, 4)

_PROGRAM = None
LAST_RESULTS = None  # test harness reads exec_time_ns / trace path from here


def _build_program():
    import concourse.bass as bass
    import concourse.bacc as bacc
    import concourse.tile as tile
    from concourse import mybir

    f32 = mybir.dt.float32
    f16 = mybir.dt.float16
    EXP = mybir.ActivationFunctionType.Exp

    nc = bacc.Bacc("TRN2", target_bir_lowering=False, debug=False,
                   num_devices=NCORES)
    # qt2: Q^T duplicated on both partition halves [128, L].
    # kt2: K^T s-chunk pairs split across partition halves:
    #   kt2[0:64, j, :] = K^T chunk 2j, kt2[64:128, j, :] = chunk 2j+1.
    qT = nc.dram_tensor("qt", [NP, 2 * E, L], f32, kind="ExternalInput").ap()
    kT = nc.dram_tensor("kt", [NP, 2 * E, NSC // 2, 128], f32,
                        kind="ExternalInput").ap()
    v = nc.dram_tensor("v", [NP, S, E], f32, kind="ExternalInput").ap()
    taus = nc.dram_tensor("taus", [1, NP], f32, kind="ExternalInput").ap()
    o = nc.dram_tensor("o", [NP, E, L], f32, kind="ExternalOutput").ap()

    with tile.TileContext(nc) as tc:
        from contextlib import ExitStack
        with ExitStack() as ctx:
            consts = ctx.enter_context(tc.tile_pool(name="consts", bufs=1))
            kq_pool = ctx.enter_context(tc.tile_pool(name="kq", bufs=2))
            v_pool = ctx.enter_context(tc.tile_pool(name="vp", bufs=2))
            exp_pool = ctx.enter_context(tc.tile_pool(name="expp", bufs=3))
            tail_pool = ctx.enter_context(tc.tile_pool(name="tail", bufs=2))
            stA_pool = ctx.enter_context(
                tc.tile_pool(name="stA", bufs=1, space="PSUM"))
            stB_pool = ctx.enter_context(
                tc.tile_pool(name="stB", bufs=1, space="PSUM"))
            ot_pool = ctx.enter_context(
                tc.tile_pool(name="ot", bufs=2, space="PSUM"))

            # tau[b] per pair, broadcast to all partitions; fold in 1/sqrt(E)
            tau_bc = consts.tile([128, NP], f32)
            nc.gpsimd.dma_start(out=tau_bc, in_=taus.to_broadcast([128, NP]))
            a_all = consts.tile([128, NP], f32)
            nc.scalar.mul(a_all, tau_bc, 1.0 / np.sqrt(float(E)))

            ones_col = consts.tile([128, 1], f32)
            nc.vector.memset(ones_col, 1.0)

            # Software-pipelined emission: PE executes in program order, so
            # QK(u+1) must be EMITTED before PV(u) — otherwise PV(u)'s wait
            # on exp(u) stalls the ready QK(u+1) behind it in the queue.
            units = []
            for p in range(NP):
                for t in range(NLT):
                    c0 = 0
                    for G in GROUPS:
                        units.append((p, t, G, c0))
                        c0 += G

            pair_tiles = {}

            def emit_loads(p):
                # fp16 operands (1 cyc/row PE stream + fast weight load);
                # gpsimd DMA casts f32 -> f16 in flight
                kt_sb = kq_pool.tile([128, NSC // 2, 128], f16, tag="kt")
                nc.gpsimd.dma_start(out=kt_sb, in_=kT[p])
                qt_sb = kq_pool.tile([128, L], f16, tag="qt")
                nc.gpsimd.dma_start(out=qt_sb, in_=qT[p])
                # V chunks [s=128, 65], col 64 = 1.0 (denominator trick)
                vp_sb = v_pool.tile([128, NSC, E + 1], f16, tag="vp")
                nc.vector.tensor_copy(vp_sb[:, :, E:E + 1],
                                      ones_col.to_broadcast([128, NSC, 1]))
                nc.gpsimd.dma_start(
                    out=vp_sb[:, :, 0:E],
                    in_=v[p].rearrange("(n q) e -> q n e", q=128))
                pair_tiles[p] = (kt_sb, qt_sb, vp_sb)

            def emit_qk(u):
                p, t, G, c0 = u
                kt_sb, qt_sb, _ = pair_tiles[p]
                pool = stA_pool if G == 4 else stB_pool
                st_ps = pool.tile([128, G * LT], f32)
                for k2 in range(G // 2):
                    j = (c0 // 2) + k2  # packed chunk-pair index
                    nc.tensor.matmul(
                        st_ps[:, (2 * k2) * LT:(2 * k2 + 1) * LT],
                        lhsT=kt_sb[0:64, j, :],
                        rhs=qt_sb[0:64, t * LT:(t + 1) * LT],
                        start=True, stop=True, tile_position=(0, 0))
                    nc.tensor.matmul(
                        st_ps[:, (2 * k2 + 1) * LT:(2 * k2 + 2) * LT],
                        lhsT=kt_sb[64:128, j, :],
                        rhs=qt_sb[64:128, t * LT:(t + 1) * LT],
                        start=True, stop=True, tile_position=(64, 0))
                return st_ps

            cur_ot = [None]

            def emit_pv(u, ex):
                p, t, G, c0 = u
                vp_sb = pair_tiles[p][2]
                if c0 == 0:
                    cur_ot[0] = ot_pool.tile([E + 1, LT], f32, name="ot_ps",
                                             tag="ot_ps")
                for k in range(G):
                    c = c0 + k
                    nc.tensor.matmul(
                        cur_ot[0],
                        lhsT=vp_sb[:, c, :],
                        rhs=ex[:, k * LT:(k + 1) * LT],
                        start=(c == 0), stop=(c == NSC - 1))

            def emit_tail(u):
                p, t, G, c0 = u
                # Normalize O^T by its denominator row, store O^T (host
                # transposes back). recip row partition-broadcast on GpSimd.
                ot_sb = tail_pool.tile([E + 1, LT], f32, tag="otsb")
                nc.vector.tensor_copy(ot_sb, cur_ot[0])
                rs_row = tail_pool.tile([1, LT], f32, tag="rsrow")
                nc.vector.reciprocal(rs_row, ot_sb[E:E + 1, :])
                rb = tail_pool.tile([64, LT], f32, tag="rb")
                nc.gpsimd.partition_broadcast(rb, rs_row, channels=64)
                on = tail_pool.tile([64, LT], f32, tag="on")
                nc.vector.tensor_mul(on, ot_sb[0:E, :], rb)
                nc.sync.dma_start(out=o[p, :, t * LT:(t + 1) * LT], in_=on)

            emit_loads(0)
            st_cur = emit_qk(units[0])
            for i, u in enumerate(units):
                p, t, G, c0 = u
                ex = exp_pool.tile([128, 4 * LT], f16, tag="ex")
                nc.scalar.activation(ex[:, 0:G * LT], st_cur, EXP,
                                     scale=a_all[:, p:p + 1])
                if i + 1 < len(units):
                    nxt = units[i + 1]
                    if nxt[0] != p:
                        emit_loads(nxt[0])
                    st_cur = emit_qk(nxt)
                emit_pv(u, ex)
                if c0 + G == NSC:  # last group of this l-tile
                    emit_tail(u)
    nc.compile()
    return nc


def _get_program():
    global _PROGRAM
    if _PROGRAM is None:
        _PROGRAM = _build_program()
    return _PROGRAM


def kernel(queries, keys, values, attn_mask=None, tau=None, delta=None):
    from concourse.bass_utils import run_bass_kernel_spmd

    queries = np.ascontiguousarray(np.asarray(queries, dtype=np.float32))
    keys = np.ascontiguousarray(np.asarray(keys, dtype=np.float32))
    values = np.ascontiguousarray(np.asarray(values, dtype=np.float32))
    tau_flat = np.asarray(tau, dtype=np.float32).reshape(B)

    # pair = b*H + h; per-pair transposed layouts (host does layout only)
    qT_base = queries.transpose(0, 2, 3, 1).reshape(B * H, E, L)
    qT_all = np.ascontiguousarray(
        np.concatenate([qT_base, qT_base], axis=1))  # [32, 128, L] duplicated
    kT_base = keys.transpose(0, 2, 3, 1).reshape(B * H, E, S)
    kc = kT_base.reshape(B * H, E, S // 128, 128)
    kT_all = np.ascontiguousarray(
        np.concatenate([kc[:, :, 0::2, :], kc[:, :, 1::2, :]], axis=1))
    # kT_all: [32, 128, 8, 128]; rows 0:64 = even chunks, 64:128 = odd
    v_all = np.ascontiguousarray(
        values.transpose(0, 2, 1, 3).reshape(B * H, S, E))

    nc = _get_program()
    in_maps = []
    for c in range(NCORES):
        lo = c * NP
        tau_pairs = np.ascontiguousarray(
            tau_flat[[(lo + i) // H for i in range(NP)]].reshape(1, NP))
        in_maps.append({
            "qt": qT_all[lo:lo + NP],
            "kt": kT_all[lo:lo + NP],
            "v": v_all[lo:lo + NP],
            "taus": tau_pairs,
        })

    kwargs = {}
    if os.environ.get("ATTN_TRACE"):
        kwargs["trace"] = True
        tmpdir = os.environ.get("ATTN_TRACE_DIR")
        if tmpdir:
            os.makedirs(tmpdir, exist_ok=True)
            kwargs["tmpdir"] = tmpdir

    res = run_bass_kernel_spmd(nc, in_maps, list(range(NCORES)), **kwargs)
    global LAST_RESULTS
    LAST_RESULTS = res

    o_all = np.concatenate([r["o"] for r in res.results], axis=0)  # [32, E, L]
    out = o_all.reshape(B, H, E, L).transpose(0, 3, 1, 2)  # [B, L, H, E]
    return np.ascontiguousarray(out)
